# revision 13
# baseline (speedup 1.0000x reference)
"""Locally-connected transposed conv (LocalConvTrans2d) on 8 TRN2 NeuronCores.

Problem: x [64,256,28,28], weight [784,256,1024] (per-location, d = oc*4*4).
  patches[b,l,d] = sum_c x[b,c,l] * weight[l,c,d]
  out[b,oc,i+di,j+dj] += patches[b,(i,j),oc,di,dj]   (fold, stride 1) -> [64,64,31,31]

v2 design (DMA-bound kernel; weight streaming dominates):
- bf16 weights + x (host cast): halves the dominant HBM traffic. rel err ~5e-3
  vs the 2e-2 gate.
- fold-in-PSUM: host reorders the weight d-dim (oc,di,dj)->(dj,di,oc) so the
  matmul for (loc j, dj) writes its N=256 (di,oc) slab into PSUM block j+dj.
  PSUM accumulation performs the dj overlap-add for free; DVE only copies
  finished blocks to SBUF (no tensor_adds, no memsets).
- 128-partition pairing: the two 7-wide quarters of each half-row run in
  lockstep on PSUM partitions 0-63 / 64-127 (tensor-engine column tiling via
  out.base_partition) -> both quarters' matmuls stream concurrently.
- strips: per half-row a [128, 10*4*64] strip (quarterA rows 0-63, quarterB
  64-127, cols = j'(10) x di(4) x oc(64)); host overlap-adds the 56 strips.

Sharding: 784 locations = 8 cores x 7 half-rows x 14 locations.
"""

import os
import sys

os.environ.setdefault("MYCRO_LOCAL_CACHE", "1")
if "/opt/trn_rl_repo" not in sys.path:
    sys.path.insert(0, "/opt/trn_rl_repo")

import numpy as np

# problem geometry (hardcoded per contract)
BS = 64           # batch
C = 256           # in channels
H = W = 28        # spatial
OC = 64           # out channels
KK = 4            # kernel size
D = OC * KK * KK  # 1024 = per-location output dim
N_CORES = 8
HRLEN = 14              # locations per half-row
QLEN = 7                # locations per quarter-row
NHR = 7                 # half-rows per core
LOC = NHR * HRLEN       # 98 locations per core
XCOLS = LOC * BS        # 6272
NBLK = QLEN + KK - 1    # 10 output-col blocks per quarter strip
BLK = KK * OC           # 256 = (di, oc) slab = one matmul's N
STRIP = NBLK * BLK      # 2560 floats per partition per strip
HOUT = H + KK - 1       # 31

_prog = None


def _build_program():
    import concourse.bass as bass
    import concourse.bacc as bacc
    import concourse.mybir as mybir
    import concourse.tile as tile
    from contextlib import ExitStack

    f32 = mybir.dt.float32
    bf16 = mybir.dt.bfloat16

    # Bacc (not raw Bass): it fuses overflow semaphore-waits into NOPs, which
    # walrus codegen's tiny per-instruction sync-wait budget requires
    nc = bacc.Bacc(trn_type="TRN2", target_bir_lowering=False, debug=False)
    xt = nc.dram_tensor("xt", [C, XCOLS], bf16, kind="ExternalInput").ap()
    # weights pre-arranged on host: [hr, j, pair, p, ch, d] with d=(dj,di,oc);
    # (ch d) innermost per partition -> 4KB contiguous DRAM runs per descriptor
    w = nc.dram_tensor("w", [NHR, QLEN, 2, 128, 2, D], bf16, kind="ExternalInput").ap()
    outp = nc.dram_tensor("outp", [128, NHR * STRIP], bf16, kind="ExternalOutput").ap()

    with ExitStack() as ctx:
        tc = ctx.enter_context(tile.TileContext(nc))
        xpool = ctx.enter_context(tc.tile_pool(name="xp", bufs=1))
        wpool = ctx.enter_context(tc.tile_pool(name="wp", bufs=2))
        strippool = ctx.enter_context(tc.tile_pool(name="sp", bufs=2))
        pspool = ctx.enter_context(tc.tile_pool(name="psp", bufs=8, space="PSUM"))

        # one 7MB weight DMA per half-row; issue hr0/hr1 up front so the
        # weight stream (the HBM-bound backbone) starts as early as possible
        wts = [None] * NHR

        def issue_w(hr):
            wt = wpool.tile([128, QLEN * 4 * D], bf16, tag="wt", name=f"wt{hr}")
            nc.sync.dma_start(
                out=wt[:].rearrange("p (j pr ch d) -> p j pr ch d",
                                    j=QLEN, pr=2, ch=2),
                in_=w[hr].rearrange("j pr p ch d -> p j pr ch d"),
            )
            wts[hr] = wt

        issue_w(0)
        issue_w(1)

        # whole x-shard resident in SBUF: [p, ch(2) x loc(98) x b(64)]
        # (scalar HWDGE ring: keeps the sync ring free for weight descriptors)
        xtile = xpool.tile([128, 2 * XCOLS], bf16)
        nc.scalar.dma_start(
            out=xtile[:].rearrange("p (ch n) -> p ch n", ch=2),
            in_=xt.rearrange("(ch p) n -> p ch n", p=128),
        )

        # dummy matmul: absorbs the x-DMA wait on the PE vector clock, so the
        # per-location matmuls below only ever wait on their own weight DMA
        ps0 = pspool.tile([128, BLK], f32, tag="blk")
        nc.tensor.matmul(
            ps0[0:64, 0:64], lhsT=xtile[:, 0:64], rhs=xtile[:, 0:64],
            start=True, stop=True,
        )

        for hr in range(NHR):
            strip = strippool.tile([128, STRIP], bf16)
            blks = [None] * NBLK

            if hr + 2 < NHR:
                issue_w(hr + 2)
            wt = wts[hr]
            wtv = wt[:].rearrange("p (j pr ch d) -> p j pr ch d", j=QLEN, pr=2, ch=2)

            for j in range(QLEN):
                jj = j
                for pair in range(2):
                    loc = hr * HRLEN + pair * QLEN + j
                    pbase = pair * 64
                    for ch in range(2):
                        lhsT = xtile[:, ch * XCOLS + loc * BS: ch * XCOLS + (loc + 1) * BS]
                        for dj in range(KK):
                            b = j + dj
                            if blks[b] is None:
                                blks[b] = pspool.tile(
                                    [128, BLK], f32, tag="blk", name=f"blk{hr}_{b}")
                            first = (j == max(0, b - (KK - 1))) and ch == 0
                            last = (j == min(QLEN - 1, b)) and ch == 1
                            nc.tensor.matmul(
                                blks[b][pbase:pbase + 64, :],
                                lhsT=lhsT,
                                rhs=wtv[:, jj, pair, ch, dj * BLK:(dj + 1) * BLK],
                                start=first, stop=last,
                                skip_group_check=True,
                            )
                # copy blocks whose contributors are all done
                done = [j] if j < QLEN - 1 else [QLEN - 1 + t for t in range(KK)]
                for b in done:
                    nc.vector.tensor_copy(strip[:, b * BLK:(b + 1) * BLK], blks[b][:])
                # ship the strip in two halves so most of it leaves early
                if j == 4:
                    nc.scalar.dma_start(
                        out=outp[:, hr * STRIP:hr * STRIP + 5 * BLK],
                        in_=strip[:, :5 * BLK])

            nc.scalar.dma_start(
                out=outp[:, hr * STRIP + 5 * BLK:(hr + 1) * STRIP],
                in_=strip[:, 5 * BLK:])
    nc.compile()
    return nc


def _get_program():
    global _prog
    if _prog is None:
        _prog = _build_program()
    return _prog


def _prep_inputs(x, weight):
    import ml_dtypes

    bf16 = ml_dtypes.bfloat16
    x = np.asarray(x, dtype=np.float32)
    weight = np.asarray(weight, dtype=np.float32)

    # x [b,c,h,w] -> xT [c, l, b] bf16
    xT = np.ascontiguousarray(
        x.reshape(BS, C, H * W).transpose(1, 2, 0)).astype(bf16)

    # weight d-dim (oc,di,dj) -> (dj,di,oc), bf16
    w2 = np.ascontiguousarray(
        weight.reshape(H * W, C, OC, KK, KK).transpose(0, 1, 4, 3, 2)
    ).reshape(H * W, C, D).astype(bf16)

    in_maps = []
    for m in range(N_CORES):
        l0 = m * LOC
        # [loc, c, d] -> [hr, pair, j, (ch p), d] -> [hr, j, pair, p, ch, d]
        wm = w2[l0:l0 + LOC].reshape(NHR, 2, QLEN, 2, 128, D).transpose(0, 2, 1, 4, 3, 5)
        in_maps.append({
            "xt": np.ascontiguousarray(xT[:, l0:l0 + LOC, :]).reshape(C, XCOLS),
            "w": np.ascontiguousarray(wm),
        })
    return in_maps


def _run(x, weight, trace=False):
    from concourse.bass_utils import run_bass_kernel_spmd

    in_maps = _prep_inputs(x, weight)
    nc = _get_program()
    br = run_bass_kernel_spmd(nc, in_maps, core_ids=list(range(N_CORES)), trace=trace)

    out = np.zeros((BS, OC, HOUT, HOUT), dtype=np.float32)
    for m in range(N_CORES):
        # [pair*64+b, hr*strip] -> [pair, b, hr, j', di, oc]
        part = np.asarray(br.results[m]["outp"], dtype=np.float32).reshape(
            2, BS, NHR, NBLK, KK, OC)
        for hr in range(NHR):
            h = NHR * m + hr
            i0 = h // 2
            j0 = HRLEN * (h % 2)
            for pair in range(2):
                c0 = j0 + pair * QLEN
                for di in range(KK):
                    out[:, :, i0 + di, c0:c0 + NBLK] += (
                        part[pair, :, hr, :, di, :].transpose(0, 2, 1))
    return out, br


def kernel(x, weight):
    out, _ = _run(x, weight)
    return out


# revision 15
# speedup vs baseline: 1.0349x; 1.0349x over previous
"""Locally-connected transposed conv (LocalConvTrans2d) on 8 TRN2 NeuronCores.

Problem: x [64,256,28,28], weight [784,256,1024] (per-location, d = oc*4*4).
  patches[b,l,d] = sum_c x[b,c,l] * weight[l,c,d]
  out[b,oc,i+di,j+dj] += patches[b,(i,j),oc,di,dj]   (fold, stride 1) -> [64,64,31,31]

v2 design (DMA-bound kernel; weight streaming dominates):
- bf16 weights + x (host cast): halves the dominant HBM traffic. rel err ~5e-3
  vs the 2e-2 gate.
- fold-in-PSUM: host reorders the weight d-dim (oc,di,dj)->(dj,di,oc) so the
  matmul for (loc j, dj) writes its N=256 (di,oc) slab into PSUM block j+dj.
  PSUM accumulation performs the dj overlap-add for free; DVE only copies
  finished blocks to SBUF (no tensor_adds, no memsets).
- 128-partition pairing: the two 7-wide quarters of each half-row run in
  lockstep on PSUM partitions 0-63 / 64-127 (tensor-engine column tiling via
  out.base_partition) -> both quarters' matmuls stream concurrently.
- strips: per half-row a [128, 10*4*64] strip (quarterA rows 0-63, quarterB
  64-127, cols = j'(10) x di(4) x oc(64)); host overlap-adds the 56 strips.

Sharding: 784 locations = 8 cores x 7 half-rows x 14 locations.
"""

import os
import sys

os.environ.setdefault("MYCRO_LOCAL_CACHE", "1")
if "/opt/trn_rl_repo" not in sys.path:
    sys.path.insert(0, "/opt/trn_rl_repo")

import numpy as np

# problem geometry (hardcoded per contract)
BS = 64           # batch
C = 256           # in channels
H = W = 28        # spatial
OC = 64           # out channels
KK = 4            # kernel size
D = OC * KK * KK  # 1024 = per-location output dim
N_CORES = 8
HRLEN = 14              # locations per half-row
QLEN = 7                # locations per quarter-row
NHR = 7                 # half-rows per core
LOC = NHR * HRLEN       # 98 locations per core
XCOLS = LOC * BS        # 6272
NBLK = QLEN + KK - 1    # 10 output-col blocks per quarter strip
BLK = KK * OC           # 256 = (di, oc) slab = one matmul's N
STRIP = NBLK * BLK      # 2560 floats per partition per strip
HOUT = H + KK - 1       # 31

_prog = None


def _build_program():
    import concourse.bass as bass
    import concourse.bacc as bacc
    import concourse.mybir as mybir
    import concourse.tile as tile
    from contextlib import ExitStack

    f32 = mybir.dt.float32
    bf16 = mybir.dt.bfloat16

    # Bacc (not raw Bass): it fuses overflow semaphore-waits into NOPs, which
    # walrus codegen's tiny per-instruction sync-wait budget requires
    nc = bacc.Bacc(trn_type="TRN2", target_bir_lowering=False, debug=False)
    xt = nc.dram_tensor("xt", [C, XCOLS], bf16, kind="ExternalInput").ap()
    # weights pre-arranged on host: [hr, j, pair, p, ch, d] with d=(dj,di,oc);
    # (ch d) innermost per partition -> 4KB contiguous DRAM runs per descriptor
    w = nc.dram_tensor("w", [NHR, QLEN, 2, 128, 2, D], bf16, kind="ExternalInput").ap()
    outp = nc.dram_tensor("outp", [128, NHR * STRIP], bf16, kind="ExternalOutput").ap()

    with ExitStack() as ctx:
        tc = ctx.enter_context(tile.TileContext(nc))
        xpool = ctx.enter_context(tc.tile_pool(name="xp", bufs=1))
        wpool = ctx.enter_context(tc.tile_pool(name="wp", bufs=3))
        strippool = ctx.enter_context(tc.tile_pool(name="sp", bufs=2))
        pspool = ctx.enter_context(tc.tile_pool(name="psp", bufs=8, space="PSUM"))

        # weight stream: 2 chunks per half-row (j 0-3 = 4MB, j 4-6 = 3MB),
        # 3 buffers, descriptor-gen issued one chunk ahead of consumption so
        # the HBM-bound weight stream never waits on descriptor generation
        CHUNKS = [(hr, j0, nj) for hr in range(NHR) for (j0, nj) in ((0, 4), (4, 3))]
        wts = [None] * len(CHUNKS)

        def issue_w(k):
            hr, j0, nj = CHUNKS[k]
            wt = wpool.tile([128, 4 * 4 * D], bf16, tag="wt", name=f"wt{k}")
            nc.sync.dma_start(
                out=wt[:, :nj * 4 * D].rearrange("p (j pr ch d) -> p j pr ch d",
                                                 j=nj, pr=2, ch=2),
                in_=w[hr, j0:j0 + nj].rearrange("j pr p ch d -> p j pr ch d"),
            )
            wts[k] = wt

        issue_w(0)
        issue_w(1)
        issue_w(2)

        # whole x-shard resident in SBUF: [p, ch(2) x loc(98) x b(64)]
        # (scalar HWDGE ring: keeps the sync ring free for weight descriptors)
        xtile = xpool.tile([128, 2 * XCOLS], bf16)
        nc.scalar.dma_start(
            out=xtile[:].rearrange("p (ch n) -> p ch n", ch=2),
            in_=xt.rearrange("(ch p) n -> p ch n", p=128),
        )

        # dummy matmul: absorbs the x-DMA wait on the PE vector clock, so the
        # per-location matmuls below only ever wait on their own weight DMA
        ps0 = pspool.tile([128, BLK], f32, tag="blk")
        nc.tensor.matmul(
            ps0[0:64, 0:64], lhsT=xtile[:, 0:64], rhs=xtile[:, 0:64],
            start=True, stop=True,
        )

        for hr in range(NHR):
            strip = strippool.tile([128, STRIP], bf16)
            blks = [None] * NBLK

            for j in range(QLEN):
                k = 2 * hr + (0 if j < 4 else 1)
                if j in (0, 4):  # top of a chunk's compute: issue chunk k+2
                    if k >= 1 and k + 2 < len(CHUNKS):
                        issue_w(k + 2)
                wt = wts[k]
                wtv = wt[:].rearrange("p (j pr ch d) -> p j pr ch d",
                                      j=4, pr=2, ch=2)
                jj = j if j < 4 else j - 4
                for pair in range(2):
                    loc = hr * HRLEN + pair * QLEN + j
                    pbase = pair * 64
                    for ch in range(2):
                        lhsT = xtile[:, ch * XCOLS + loc * BS: ch * XCOLS + (loc + 1) * BS]
                        for dj in range(KK):
                            b = j + dj
                            if blks[b] is None:
                                blks[b] = pspool.tile(
                                    [128, BLK], f32, tag="blk", name=f"blk{hr}_{b}")
                            first = (j == max(0, b - (KK - 1))) and ch == 0
                            last = (j == min(QLEN - 1, b)) and ch == 1
                            nc.tensor.matmul(
                                blks[b][pbase:pbase + 64, :],
                                lhsT=lhsT,
                                rhs=wtv[:, jj, pair, ch, dj * BLK:(dj + 1) * BLK],
                                start=first, stop=last,
                                skip_group_check=True,
                            )
                # copy blocks whose contributors are all done
                done = [j] if j < QLEN - 1 else [QLEN - 1 + t for t in range(KK)]
                for b in done:
                    nc.vector.tensor_copy(strip[:, b * BLK:(b + 1) * BLK], blks[b][:])
                # ship the strip in two halves so most of it leaves early
                if j == 4:
                    nc.scalar.dma_start(
                        out=outp[:, hr * STRIP:hr * STRIP + 5 * BLK],
                        in_=strip[:, :5 * BLK])

            nc.scalar.dma_start(
                out=outp[:, hr * STRIP + 5 * BLK:(hr + 1) * STRIP],
                in_=strip[:, 5 * BLK:])
    nc.compile()
    return nc


def _get_program():
    global _prog
    if _prog is None:
        _prog = _build_program()
    return _prog


def _prep_inputs(x, weight):
    import ml_dtypes

    bf16 = ml_dtypes.bfloat16
    x = np.asarray(x, dtype=np.float32)
    weight = np.asarray(weight, dtype=np.float32)

    # x [b,c,h,w] -> xT [c, l, b] bf16
    xT = np.ascontiguousarray(
        x.reshape(BS, C, H * W).transpose(1, 2, 0)).astype(bf16)

    # weight d-dim (oc,di,dj) -> (dj,di,oc), bf16
    w2 = np.ascontiguousarray(
        weight.reshape(H * W, C, OC, KK, KK).transpose(0, 1, 4, 3, 2)
    ).reshape(H * W, C, D).astype(bf16)

    in_maps = []
    for m in range(N_CORES):
        l0 = m * LOC
        # [loc, c, d] -> [hr, pair, j, (ch p), d] -> [hr, j, pair, p, ch, d]
        wm = w2[l0:l0 + LOC].reshape(NHR, 2, QLEN, 2, 128, D).transpose(0, 2, 1, 4, 3, 5)
        in_maps.append({
            "xt": np.ascontiguousarray(xT[:, l0:l0 + LOC, :]).reshape(C, XCOLS),
            "w": np.ascontiguousarray(wm),
        })
    return in_maps


def _run(x, weight, trace=False):
    from concourse.bass_utils import run_bass_kernel_spmd

    in_maps = _prep_inputs(x, weight)
    nc = _get_program()
    br = run_bass_kernel_spmd(nc, in_maps, core_ids=list(range(N_CORES)), trace=trace)

    out = np.zeros((BS, OC, HOUT, HOUT), dtype=np.float32)
    for m in range(N_CORES):
        # [pair*64+b, hr*strip] -> [pair, b, hr, j', di, oc]
        part = np.asarray(br.results[m]["outp"], dtype=np.float32).reshape(
            2, BS, NHR, NBLK, KK, OC)
        for hr in range(NHR):
            h = NHR * m + hr
            i0 = h // 2
            j0 = HRLEN * (h % 2)
            for pair in range(2):
                c0 = j0 + pair * QLEN
                for di in range(KK):
                    out[:, :, i0 + di, c0:c0 + NBLK] += (
                        part[pair, :, hr, :, di, :].transpose(0, 2, 1))
    return out, br


def kernel(x, weight):
    out, _ = _run(x, weight)
    return out


# revision 19
# speedup vs baseline: 1.1371x; 1.0987x over previous
"""Locally-connected transposed conv (LocalConvTrans2d) on 8 TRN2 NeuronCores.

Problem: x [64,256,28,28], weight [784,256,1024] (per-location, d = oc*4*4).
  patches[b,l,d] = sum_c x[b,c,l] * weight[l,c,d]
  out[b,oc,i+di,j+dj] += patches[b,(i,j),oc,di,dj]   (fold, stride 1) -> [64,64,31,31]

v2 design (DMA-bound kernel; weight streaming dominates):
- bf16 weights + x (host cast): halves the dominant HBM traffic. rel err ~5e-3
  vs the 2e-2 gate.
- fold-in-PSUM: host reorders the weight d-dim (oc,di,dj)->(dj,di,oc) so the
  matmul for (loc j, dj) writes its N=256 (di,oc) slab into PSUM block j+dj.
  PSUM accumulation performs the dj overlap-add for free; DVE only copies
  finished blocks to SBUF (no tensor_adds, no memsets).
- 128-partition pairing: the two 7-wide quarters of each half-row run in
  lockstep on PSUM partitions 0-63 / 64-127 (tensor-engine column tiling via
  out.base_partition) -> both quarters' matmuls stream concurrently.
- strips: per half-row a [128, 10*4*64] strip (quarterA rows 0-63, quarterB
  64-127, cols = j'(10) x di(4) x oc(64)); host overlap-adds the 56 strips.

Sharding: 784 locations = 8 cores x 7 half-rows x 14 locations.
"""

import os
import sys

os.environ.setdefault("MYCRO_LOCAL_CACHE", "1")
if "/opt/trn_rl_repo" not in sys.path:
    sys.path.insert(0, "/opt/trn_rl_repo")

import numpy as np

# problem geometry (hardcoded per contract)
BS = 64           # batch
C = 256           # in channels
H = W = 28        # spatial
OC = 64           # out channels
KK = 4            # kernel size
D = OC * KK * KK  # 1024 = per-location output dim
N_CORES = 8
HRLEN = 14              # locations per half-row
QLEN = 7                # locations per quarter-row
NHR = 7                 # half-rows per core
LOC = NHR * HRLEN       # 98 locations per core
XCOLS = LOC * BS        # 6272
NBLK = QLEN + KK - 1    # 10 output-col blocks per quarter strip
BLK = KK * OC           # 256 = (di, oc) slab = one matmul's N
STRIP = NBLK * BLK      # 2560 floats per partition per strip
HOUT = H + KK - 1       # 31

_prog = None


def _build_program():
    import concourse.bass as bass
    import concourse.bacc as bacc
    import concourse.mybir as mybir
    import concourse.tile as tile
    from contextlib import ExitStack

    f32 = mybir.dt.float32
    bf16 = mybir.dt.bfloat16

    # Bacc (not raw Bass): it fuses overflow semaphore-waits into NOPs, which
    # walrus codegen's tiny per-instruction sync-wait budget requires
    nc = bacc.Bacc(trn_type="TRN2", target_bir_lowering=False, debug=False)
    xt = nc.dram_tensor("xt", [C, XCOLS], bf16, kind="ExternalInput").ap()
    # weights pre-arranged on host: [hr, j, pair, p, ch, d] with d=(dj,di,oc);
    # (ch d) innermost per partition -> 4KB contiguous DRAM runs per descriptor
    w = nc.dram_tensor("w", [NHR, QLEN, 2, 128, 2, D], bf16, kind="ExternalInput").ap()
    outp = nc.dram_tensor("outp", [128, NHR * STRIP], bf16, kind="ExternalOutput").ap()

    with ExitStack() as ctx:
        tc = ctx.enter_context(tile.TileContext(nc))
        xpool = ctx.enter_context(tc.tile_pool(name="xp", bufs=1))
        wpool = ctx.enter_context(tc.tile_pool(name="wp", bufs=2))
        strippool = ctx.enter_context(tc.tile_pool(name="sp", bufs=2))
        pspool = ctx.enter_context(tc.tile_pool(name="psp", bufs=8, space="PSUM"))

        # ALL DMAs go on the sync HWDGE ring: the ring is FIFO, so each weight
        # chunk's data completes (and its sem fires) as early as possible --
        # a second ring would interleave packets and dilute chunk completion.
        wts = [None] * NHR

        def issue_w(hr):
            wt = wpool.tile([128, QLEN * 4 * D], bf16, tag="wt", name=f"wt{hr}")
            nc.sync.dma_start(
                out=wt[:].rearrange("p (j pr ch d) -> p j pr ch d",
                                    j=QLEN, pr=2, ch=2),
                in_=w[hr].rearrange("j pr p ch d -> p j pr ch d"),
            )
            wts[hr] = wt

        # whole x-shard resident in SBUF: [p, ch(2) x loc(98) x b(64)]
        xtile = xpool.tile([128, 2 * XCOLS], bf16)
        nc.sync.dma_start(
            out=xtile[:].rearrange("p (ch n) -> p ch n", ch=2),
            in_=xt.rearrange("(ch p) n -> p ch n", p=128),
        )
        issue_w(0)
        issue_w(1)

        # dummy matmul: absorbs the x-DMA wait on the PE vector clock, so the
        # per-location matmuls below only ever wait on their own weight DMA
        ps0 = pspool.tile([128, BLK], f32, tag="blk")
        nc.tensor.matmul(
            ps0[0:64, 0:64], lhsT=xtile[:, 0:64], rhs=xtile[:, 0:64],
            start=True, stop=True,
        )

        for hr in range(NHR):
            strip = strippool.tile([128, STRIP], bf16)
            blks = [None] * NBLK

            # issue w(hr+2) BEFORE this iter's strip DMA in program order, so
            # its descriptor-gen is gated only by consumed(hr), never by the
            # strip's copy chain
            if hr + 2 < NHR:
                issue_w(hr + 2)
            wt = wts[hr]
            wtv = wt[:].rearrange("p (j pr ch d) -> p j pr ch d",
                                  j=QLEN, pr=2, ch=2)

            for j in range(QLEN):
                jj = j
                for pair in range(2):
                    loc = hr * HRLEN + pair * QLEN + j
                    pbase = pair * 64
                    for ch in range(2):
                        lhsT = xtile[:, ch * XCOLS + loc * BS: ch * XCOLS + (loc + 1) * BS]
                        for dj in range(KK):
                            b = j + dj
                            if blks[b] is None:
                                blks[b] = pspool.tile(
                                    [128, BLK], f32, tag="blk", name=f"blk{hr}_{b}")
                            first = (j == max(0, b - (KK - 1))) and ch == 0
                            last = (j == min(QLEN - 1, b)) and ch == 1
                            nc.tensor.matmul(
                                blks[b][pbase:pbase + 64, :],
                                lhsT=lhsT,
                                rhs=wtv[:, jj, pair, ch, dj * BLK:(dj + 1) * BLK],
                                start=first, stop=last,
                                skip_group_check=True,
                            )
                # copy blocks whose contributors are all done
                done = [j] if j < QLEN - 1 else [QLEN - 1 + t for t in range(KK)]
                for b in done:
                    nc.vector.tensor_copy(strip[:, b * BLK:(b + 1) * BLK], blks[b][:])
                # ship the strip in two halves so most of it leaves early
                if j == 4:
                    nc.sync.dma_start(
                        out=outp[:, hr * STRIP:hr * STRIP + 5 * BLK],
                        in_=strip[:, :5 * BLK])

            nc.sync.dma_start(
                out=outp[:, hr * STRIP + 5 * BLK:(hr + 1) * STRIP],
                in_=strip[:, 5 * BLK:])
    nc.compile()
    return nc


def _get_program():
    global _prog
    if _prog is None:
        _prog = _build_program()
    return _prog


def _prep_inputs(x, weight):
    import ml_dtypes

    bf16 = ml_dtypes.bfloat16
    x = np.asarray(x, dtype=np.float32)
    weight = np.asarray(weight, dtype=np.float32)

    # x [b,c,h,w] -> xT [c, l, b] bf16
    xT = np.ascontiguousarray(
        x.reshape(BS, C, H * W).transpose(1, 2, 0)).astype(bf16)

    # weight d-dim (oc,di,dj) -> (dj,di,oc), bf16
    w2 = np.ascontiguousarray(
        weight.reshape(H * W, C, OC, KK, KK).transpose(0, 1, 4, 3, 2)
    ).reshape(H * W, C, D).astype(bf16)

    in_maps = []
    for m in range(N_CORES):
        l0 = m * LOC
        # [loc, c, d] -> [hr, pair, j, (ch p), d] -> [hr, j, pair, p, ch, d]
        wm = w2[l0:l0 + LOC].reshape(NHR, 2, QLEN, 2, 128, D).transpose(0, 2, 1, 4, 3, 5)
        in_maps.append({
            "xt": np.ascontiguousarray(xT[:, l0:l0 + LOC, :]).reshape(C, XCOLS),
            "w": np.ascontiguousarray(wm),
        })
    return in_maps


def _run(x, weight, trace=False):
    from concourse.bass_utils import run_bass_kernel_spmd

    in_maps = _prep_inputs(x, weight)
    nc = _get_program()
    br = run_bass_kernel_spmd(nc, in_maps, core_ids=list(range(N_CORES)), trace=trace)

    out = np.zeros((BS, OC, HOUT, HOUT), dtype=np.float32)
    for m in range(N_CORES):
        # [pair*64+b, hr*strip] -> [pair, b, hr, j', di, oc]
        part = np.asarray(br.results[m]["outp"], dtype=np.float32).reshape(
            2, BS, NHR, NBLK, KK, OC)
        for hr in range(NHR):
            h = NHR * m + hr
            i0 = h // 2
            j0 = HRLEN * (h % 2)
            for pair in range(2):
                c0 = j0 + pair * QLEN
                for di in range(KK):
                    out[:, :, i0 + di, c0:c0 + NBLK] += (
                        part[pair, :, hr, :, di, :].transpose(0, 2, 1))
    return out, br


def kernel(x, weight):
    out, _ = _run(x, weight)
    return out
